# revision 3
# baseline (speedup 1.0000x reference)
"""CRF log_prob kernel for Trainium2 (8 NeuronCores, Bass/Tile).

Strategy: log_scores (gold path) is computed exactly on host in fp64 (cheap
gathers). The expensive part, log_partitions (forward algorithm over T=1024
steps), runs on-device via a chunked parallel-in-time scan:

 - Work in the probability domain with host-normalized emission factors
   e_t = exp(em_t - logsumexp(em_t)); one recursion step is then
   z <- (E^T z) * e_t, a 128x128 matmul (two 64-state conjugates packed
   block-diagonally) followed by an elementwise multiply.
 - T is split into 32 chunks of 32 steps processed in parallel as matmul
   columns. Chunk-boundary states are obtained from a short warm-up window
   (W steps from a uniform state): the CRF step contracts in the Hilbert
   metric with rate kappa = tanh(diam(trans)/4), so the boundary error is
   kappa^W (checked against the actual transitions at runtime; fallback to
   an exact numpy path if the contraction is too weak).
 - Per-step per-state sums c_t = sum(z) and tail dots d_t = z . exp(tail)
   are extracted with a second small matmul; host reassembles exact
   log-partition values from log d at t = len-1 plus a per-chunk scale
   telescoping (Lambda) and the host-side shift sums (all fp64).

Sharding: core k of 8 handles conjugate pair k%4 (2 of 8 "C" channels,
packed on the 128 partitions) and batch half k//4 (32 of 64 sequences).
"""
import os
import sys

for _p in ("/opt/trn_rl_repo", "/root/.axon_site", "/root/.axon_site/_ro/pypackages"):
    if _p not in sys.path and os.path.isdir(_p):
        sys.path.append(_p)

import numpy as np
import ml_dtypes

T, B, C, N = 1024, 64, 8, 64
NCH, S = 32, 32          # chunks x steps per chunk (NCH*S == T)
BCORE, FH = 32, 512      # batch per core, columns per half
NCORE = 8
GRP = 3                  # sums-steps packed per PSUM bank (offsets 0/32/64)
NG2 = (S + GRP - 1) // GRP   # pass-2 sum groups (11)
NWCOL = (NCH - 1) * BCORE    # window columns (992)

_BF = ml_dtypes.bfloat16
_CACHE = {}


# ---------------------------------------------------------------- device ---

def _build_nc(W):
    from concourse import bacc
    import concourse.tile as tile
    from concourse import mybir

    BF16 = mybir.dt.bfloat16
    F32 = mybir.dt.float32

    nc = bacc.Bacc(None, target_bir_lowering=False)
    em_d = nc.declare_dram_parameter("em", (S, 2, 128, FH), BF16, isOutput=False)
    ee_d = nc.declare_dram_parameter("ee", (128, 128), BF16, isOutput=False)
    sw_d = nc.declare_dram_parameter("sums_w", (128, 4), BF16, isOutput=False)
    z0_d = nc.declare_dram_parameter("z0init", (128, BCORE), BF16, isOutput=False)
    # cdout[h, g, rows, col]: g in 0..NG2-1 pass-2 groups (step s=3g+j at rows
    # 32j..32j+3: [c_c0, c_c1, d_c0, d_c1]), g == NG2 is the window sigma
    # group (rows 0..3).
    cd_d = nc.declare_dram_parameter("cdout", (2, NG2 + 1, 68, FH), F32,
                                     isOutput=True)

    PF = 4  # streamed-em prefetch distance

    with tile.TileContext(nc) as tc:
        with (
            tc.tile_pool(name="consts", bufs=1) as consts,
            tc.tile_pool(name="winem", bufs=1) as winem,
            tc.tile_pool(name="empool", bufs=6) as empool,
            tc.tile_pool(name="zpool", bufs=3) as zpool,
            tc.tile_pool(name="stage", bufs=3) as stage,
            tc.tile_pool(name="psum_m", bufs=2, space="PSUM") as psum_m,
            tc.tile_pool(name="psum_s", bufs=2, space="PSUM") as psum_s,
        ):
            ee_sb = consts.tile([128, 128], BF16)
            nc.sync.dma_start(ee_sb[:], ee_d[:])
            sw_sb = consts.tile([128, 4], BF16)
            nc.sync.dma_start(sw_sb[:], sw_d[:])
            z0_sb = consts.tile([128, BCORE], BF16)
            nc.sync.dma_start(z0_sb[:], z0_d[:])

            # window-resident em slices (also used by pass-2 steps S-W..S-1)
            res_em = {}
            for s in range(S - W, S):
                for h in range(2):
                    t = winem.tile([128, FH], BF16, tag=f"wem{s}_{h}", name=f"wem{s}_{h}")
                    nc.sync.dma_start(t[:], em_d[s, h])
                    res_em[(s, h)] = t

            stream_em = {}

            def load_stream(s):
                for h in range(2):
                    t = empool.tile([128, FH], BF16, tag=f"em{h}", name=f"ems{s}_{h}")
                    nc.sync.dma_start(t[:], em_d[s, h])
                    stream_em[(s, h)] = t

            for s in range(min(PF, S - W)):
                load_stream(s)

            ncols = (FH, NWCOL - FH)  # valid columns per half in the window

            # ---- pass 1: warm-up windows for chunk boundaries 1..31 ----
            wz = []
            for h in range(2):
                t = zpool.tile([128, FH], BF16, tag=f"z{h}", name=f"wz{h}")
                nc.vector.memset(t[:], 1.0 / N)
                wz.append(t)
            for s in range(S - W, S):
                for h in range(2):
                    nc_h = ncols[h]
                    ps = psum_m.tile([128, FH], F32, tag=f"mm{h}", name=f"wps{s}_{h}")
                    nc.tensor.matmul(ps[:, :nc_h], ee_sb[:], wz[h][:, :nc_h],
                                     start=True, stop=True)
                    zn = zpool.tile([128, FH], BF16, tag=f"z{h}", name=f"wzn{s}_{h}")
                    nc.vector.tensor_mul(zn[:, :nc_h], ps[:, :nc_h],
                                         res_em[(s, h)][:, :nc_h])
                    wz[h] = zn

            # sigma = column sums of the window-final states
            for h in range(2):
                nc_h = ncols[h]
                ps2 = psum_s.tile([128, FH], F32, tag=f"sums{h}", name=f"sig{h}")
                nc.tensor.matmul(ps2[0:4, :nc_h], sw_sb[:], wz[h][:, :nc_h],
                                 start=True, stop=True)
                st = stage.tile([68, FH], F32, tag=f"st{h}", name=f"sigst{h}")
                nc.vector.tensor_copy(st[:], ps2[0:68, :])
                nc.sync.dma_start(cd_d[h, NG2], st[:])

            # ---- pass 2 init: chunk 0 = one-hot trick, chunks 1.. = shifted
            # window-final states ----
            z = []
            for h in range(2):
                t = zpool.tile([128, FH], BF16, tag=f"z{h}", name=f"zinit{h}")
                z.append(t)
            nc.vector.tensor_copy(z[0][:, 0:BCORE], z0_sb[:])
            nc.vector.tensor_copy(z[0][:, BCORE:FH], wz[0][:, 0:FH - BCORE])
            nc.vector.tensor_copy(z[1][:, 0:BCORE], wz[0][:, FH - BCORE:FH])
            nc.vector.tensor_copy(z[1][:, BCORE:FH], wz[1][:, 0:FH - BCORE])

            # ---- pass 2: 32 steps over all chunks ----
            ps2 = [None, None]
            for s in range(S):
                if s + PF < S - W:
                    load_stream(s + PF)
                g, j = divmod(s, GRP)
                for h in range(2):
                    emt = res_em[(s, h)] if s >= S - W else stream_em.pop((s, h))
                    ps = psum_m.tile([128, FH], F32, tag=f"mm{h}", name=f"ps{s}_{h}")
                    nc.tensor.matmul(ps[:], ee_sb[:], z[h][:], start=True,
                                     stop=True)
                    zn = zpool.tile([128, FH], BF16, tag=f"z{h}", name=f"zn{s}_{h}")
                    nc.vector.tensor_mul(zn[:], ps[:], emt[:])
                    z[h] = zn
                    if j == 0:
                        ps2[h] = psum_s.tile([128, FH], F32, tag=f"sums{h}", name=f"ps2_{s}_{h}")
                    nc.tensor.matmul(ps2[h][32 * j:32 * j + 4, :], sw_sb[:],
                                     z[h][:], start=True, stop=True)
                    if j == GRP - 1 or s == S - 1:
                        st = stage.tile([68, FH], F32, tag=f"st{h}", name=f"st{s}_{h}")
                        nc.vector.tensor_copy(st[:], ps2[h][0:68, :])
                        nc.sync.dma_start(cd_d[h, g], st[:])
    nc.compile()
    return nc


def _get_nc(W):
    key = ("nc", W)
    if key not in _CACHE:
        _CACHE[key] = _build_nc(W)
    return _CACHE[key]


# ------------------------------------------------------------------ host ---

def _host_prep(em, tr, hd, tl):
    """-> EMs (per-core [S,2,128,FH] bf16), EEs, SUMSs (per c-pair), Z0INIT,
    r [T,B,C] f64."""
    em = np.asarray(em, np.float32)
    tr32 = np.asarray(tr, np.float32)

    m = em.max(-1, keepdims=True)
    lse = m[..., 0] + np.log(np.exp(em - m).sum(-1))     # [T,B,C]
    r = lse.astype(np.float64)
    expem = np.exp(em - lse[..., None])

    a0 = np.asarray(hd, np.float32)[None] + em[0]         # [B,C,N]
    m0 = a0.max(-1, keepdims=True)
    r0 = m0[..., 0] + np.log(np.exp(a0 - m0).sum(-1))     # [B,C]
    expem[0] = np.exp(a0 - r0[..., None] - tr32[None, :, 0, :])
    r[0] = r0.astype(np.float64)

    expem = expem.astype(_BF)

    e5 = expem.reshape(NCH, S, B, C, N)
    EMs = []
    for k in range(NCORE):
        cp, bh = k % 4, k // 4
        sub = e5[:, :, BCORE * bh:BCORE * (bh + 1), 2 * cp:2 * cp + 2, :]
        x = np.ascontiguousarray(sub.transpose(1, 3, 4, 0, 2)).reshape(
            S, 128, NCH * BCORE)
        EMs.append(np.ascontiguousarray(
            np.stack([x[:, :, :FH], x[:, :, FH:]], axis=1)))

    EEs, SUMSs = [], []
    tl32 = np.asarray(tl, np.float32)
    for cp in range(4):
        EE = np.zeros((128, 128), np.float32)
        EE[:64, :64] = np.exp(tr32[2 * cp])
        EE[64:, 64:] = np.exp(tr32[2 * cp + 1])
        SUMS = np.zeros((128, 4), np.float32)
        SUMS[:64, 0] = 1.0
        SUMS[64:, 1] = 1.0
        SUMS[:64, 2] = np.exp(tl32[2 * cp])
        SUMS[64:, 3] = np.exp(tl32[2 * cp + 1])
        EEs.append(EE.astype(_BF))
        SUMSs.append(SUMS.astype(_BF))

    Z0INIT = np.zeros((128, BCORE), np.float32)
    Z0INIT[0, :] = 1.0
    Z0INIT[64, :] = 1.0
    return EMs, EEs, SUMSs, Z0INIT.astype(_BF), r


def _assemble(cd_by_core, r, lengths):
    """cd_by_core: [NCORE] arrays [2, NG2+1, 68, FH] -> logZ [B,C] f64."""
    lengths = np.asarray(lengths).astype(np.int64)
    # Unpack to CC/DD [NCH, S, B, C] and SIG [NCH, B, C] (SIG[0] unused = 1).
    CC = np.empty((NCH, S, B, C), np.float64)
    DD = np.empty((NCH, S, B, C), np.float64)
    SIG = np.ones((NCH, B, C), np.float64)
    for k in range(NCORE):
        cp, bh = k % 4, k // 4
        cd = cd_by_core[k].astype(np.float64)  # [2, NG2+1, 68, FH]
        flat = np.concatenate([cd[0], cd[1]], axis=-1)  # [NG2+1, 68, 1024]
        bs = slice(BCORE * bh, BCORE * (bh + 1))
        for s in range(S):
            g, j = divmod(s, GRP)
            rows = flat[g, 32 * j:32 * j + 4, :]  # [4, 1024]
            grid = rows.reshape(4, NCH, BCORE)
            for cl in range(2):
                CC[:, s, bs, 2 * cp + cl] = grid[cl]
                DD[:, s, bs, 2 * cp + cl] = grid[2 + cl]
        sig = flat[NG2, 0:4, :NWCOL].reshape(4, NCH - 1, BCORE)
        for cl in range(2):
            SIG[1:, bs, 2 * cp + cl] = sig[cl]

    ratios = np.log(CC[:-1, S - 1]) - np.log(SIG[1:])    # [NCH-1, B, C]
    LAM = np.zeros((NCH, B, C), np.float64)
    LAM[1:] = np.cumsum(ratios, axis=0)

    rcum = np.cumsum(r, axis=0)                          # [T,B,C]
    istar = (lengths - 1) // S                           # [B]
    sstar = (lengths - 1) % S
    bidx = np.arange(B)
    logZ = (LAM[istar, bidx, :]
            + np.log(DD[istar, sstar, bidx, :])
            + rcum[lengths - 1, bidx, :])
    return logZ


def _gold_scores(em, tags, lengths, tr, hd, tl):
    em = np.asarray(em, np.float64)
    tags = np.asarray(tags).astype(np.int64)
    lengths = np.asarray(lengths).astype(np.int64)
    tr = np.asarray(tr, np.float64)
    hd = np.asarray(hd, np.float64)
    tl = np.asarray(tl, np.float64)
    Tn, Bn, Cn, Nn = em.shape
    maskf = (np.arange(Tn)[:, None] < lengths[None, :]).astype(np.float64)
    c_idx = np.arange(Cn)
    em_score = np.take_along_axis(em, tags[..., None], axis=-1)[..., 0]
    em_total = (em_score * maskf[:, :, None]).sum(0)
    head_sc = hd[c_idx[None, :], tags[0]]
    tags_last = tags[lengths - 1, np.arange(Bn)]
    tail_sc = tl[c_idx[None, :], tags_last]
    trans_sc = tr[c_idx[None, None, :], tags[:-1], tags[1:]]
    trans_total = (trans_sc * maskf[1:, :, None]).sum(0)
    return em_total + head_sc + tail_sc + trans_total


def _numpy_fallback(emissions, tags, lengths, transitions, head_transitions,
                    tail_transitions):
    em = np.asarray(emissions, np.float64)
    lengths = np.asarray(lengths).astype(np.int64)
    tr = np.asarray(transitions, np.float64)
    hd = np.asarray(head_transitions, np.float64)
    tl = np.asarray(tail_transitions, np.float64)
    Tn, Bn, Cn, Nn = em.shape
    mask = np.arange(Tn)[:, None] < lengths[None, :]
    alpha = hd[None, :, :] + em[0]
    E = np.exp(tr)
    for t in range(1, Tn):
        mb = mask[t]
        if not mb.any():
            break
        mx = alpha.max(-1, keepdims=True)
        s = np.einsum("bcn,cnm->bcm", np.exp(alpha - mx), E, optimize=True)
        nxt = mx + np.log(s) + em[t]
        alpha = np.where(mb[:, None, None], nxt, alpha)
    am = alpha + tl[None, :, :]
    mz = am.max(-1, keepdims=True)
    logZ = mz[..., 0] + np.log(np.exp(am - mz).sum(-1))
    ls = _gold_scores(em, tags, lengths, tr, hd, tl)
    return (ls - logZ).astype(np.float32)


def _window_len(tr):
    """Required warm-up length from the actual transition contraction rate."""
    tr = np.asarray(tr, np.float64)
    diam = 0.0
    for c in range(tr.shape[0]):
        t = tr[c]
        d = t[:, None, :] - t[None, :, :]        # [n, n', m]
        diam = max(diam, float((d.max(-1) - d.min(-1)).max()))
    kappa = np.tanh(diam / 4.0)
    if kappa < 1e-9:
        return 4
    Wn = int(np.ceil(np.log(1e-4) / np.log(kappa))) + 1
    return max(4, Wn)


def kernel(emissions, tags, lengths, transitions, head_transitions,
           tail_transitions):
    em = np.asarray(emissions)
    ok = (em.shape == (T, B, C, N)
          and np.asarray(transitions).shape == (C, N, N))
    if ok:
        Wn = _window_len(transitions)
        ok = Wn <= S - 1
    if not ok:
        return _numpy_fallback(emissions, tags, lengths, transitions,
                               head_transitions, tail_transitions)
    try:
        from concourse import bass2jax
        EMs, EEs, SUMSs, Z0INIT, r = _host_prep(
            emissions, transitions, head_transitions, tail_transitions)
        nc = _get_nc(Wn)
        in_maps = [{
            "em": EMs[k],
            "ee": EEs[k % 4],
            "sums_w": SUMSs[k % 4],
            "z0init": Z0INIT,
        } for k in range(NCORE)]
        results = bass2jax.run_bass_via_pjrt(nc, in_maps, n_cores=NCORE)
        cd_by_core = [results[k]["cdout"] for k in range(NCORE)]
        logZ = _assemble(cd_by_core, r, lengths)
        if not np.all(np.isfinite(logZ)):
            raise FloatingPointError("non-finite log partitions")
        ls = _gold_scores(emissions, tags, lengths, transitions,
                          head_transitions, tail_transitions)
        return (ls - logZ).astype(np.float32)
    except Exception:
        if os.environ.get("CRF_NO_FALLBACK"):
            raise
        import traceback
        traceback.print_exc()
        return _numpy_fallback(emissions, tags, lengths, transitions,
                               head_transitions, tail_transitions)
